# revision 11
# baseline (speedup 1.0000x reference)
"""Distributed Trainium2 kernel for nn_Attention (dense transformer block:
fused QKV projection + per-head RMSNorm + rotary + causal GQA attention + output
projection), running SPMD on 8 NeuronCores.

Sharding (rank-uniform, no divergent control flow):
  - 8 cores = 2 batch groups x 4 tensor-parallel ranks.
  - Core c: batch b = c // 4, rank r = c % 4.
  - QKV projection + attention are head-sharded: core r computes q heads
    4r..4r+3 and kv head r for ALL tokens of its batch (wqkv column slice is
    per-core input data, so the compiled graph is identical on every core).
  - Every core runs the same causal tile sweep -> perfect load balance.
  - One AllGather (2MB bf16) per head re-shards y from head-split to
    token-split, then the output projection runs locally with the full
    contraction dim (no all-reduce).

Layout tricks:
  - Host pre-transposes x, wqkv, wo so the kernel's matmuls need no on-device
    transposes (except tiny 128x128 PE transposes for V).
  - Rope pair-swap is free: the head_dim rows of wqkv (and the norm weights /
    cos/sin tables) are host-permuted to [evens, odds] order, so the rotary
    "rotate half" is a 64-partition swap done by one DVE stream_shuffle.
  - Scores are computed transposed [kv, q]; exp is fused into the PSUM->SBUF
    eviction on the ScalarEngine; the softmax denominator is accumulated by
    the DVE across kv blocks and reduced by one tiny f32r ones-matmul per
    (head, q-chunk) -- not one 512-cycle matmul per kv block.
  - Causal diagonal blocks only compute the unmasked column range.
  - RMSNorm reduces to a per-token scalar computed with a ones-matmul over
    the squared tile; the 1/sqrt(head_dim) score scale folds into the q-side
    scalar.
  - wo is host-relaid so every output-tile weight panel is one contiguous
    4KB-per-partition DMA, staged fully into SBUF during phase 1/2.
  - All big matmuls run in bf16 with f32 PSUM accumulation.
"""

import numpy as np
import ml_dtypes

import concourse.bass as bass
import concourse.bass_isa as bass_isa
import concourse.mybir as mybir
import concourse.tile as tile
from concourse import bacc
from concourse.bass_utils import run_bass_kernel_spmd

BF16 = mybir.dt.bfloat16
F32 = mybir.dt.float32
F32R = mybir.dt.float32r

DIM = 2048
NH = 16
NKV = 4
HD = 128
EPS = 1e-5
N_CORES = 8
RG = [[0, 1, 2, 3], [4, 5, 6, 7]]  # per-batch tensor-parallel groups

HL = NH // NKV  # q heads per core (= GQA group size) = 4
EW = HL * HD + 2 * HD  # wqkv column-slice width per core = 768
NDT = DIM // 128  # contraction tiles = 16

# per-32-partition-quadrant permutation that swaps 16-partition halves
SWAP_MASK = list(range(16, 32)) + list(range(16))

DIAG_SLICE = True  # only compute unmasked columns of causal-diagonal blocks


def build_graph(S):
    """Build + compile the SPMD graph for sequence length S. Returns nc."""
    TPT = S // 4       # tokens per core after the gather (output rows per core)
    TCW = S // 4       # token chunk width for phase 1 (moving dim <= 512)
    NTT = S // TCW     # number of token chunks = 4
    QC = 512           # attention q-chunk width
    KB = 128           # kv block size
    NQC = S // QC      # q chunks per head
    NB = S // 128      # 128-token blocks (for V layout)

    nc = bacc.Bacc("TRN2", target_bir_lowering=False, debug=False,
                   num_devices=N_CORES)

    # ---- DRAM I/O ----
    xT_d = nc.dram_tensor("xT", [DIM, S], BF16, kind="ExternalInput")
    w_d = nc.dram_tensor("wslice", [DIM, EW], BF16, kind="ExternalInput")
    wo_d = nc.dram_tensor("woS", [128, NDT * NDT * 128], BF16,
                          kind="ExternalInput")
    cos_d = nc.dram_tensor("cosF", [128, S], F32, kind="ExternalInput")
    sin_d = nc.dram_tensor("sinF", [128, S], F32, kind="ExternalInput")
    idn_d = nc.dram_tensor("ident", [128, 128], BF16, kind="ExternalInput")
    msk_d = nc.dram_tensor("masks", [KB, (QC // KB) * QC], BF16, kind="ExternalInput")
    qw_d = nc.dram_tensor("qw", [128, 1], F32, kind="ExternalInput")
    kw_d = nc.dram_tensor("kw", [128, 1], F32, kind="ExternalInput")
    out_d = nc.dram_tensor("out", [DIM, TPT], BF16, kind="ExternalOutput")

    with tile.TileContext(nc) as tc:
        with tc.tile_pool(name="const", bufs=1) as cpool, \
             tc.tile_pool(name="wo_sb", bufs=1) as wopool, \
             tc.tile_pool(name="big", bufs=1) as bigpool, \
             tc.tile_pool(name="dram", bufs=1, space="DRAM") as dpool:

            # constants
            idn = cpool.tile([128, 128], BF16, tag="idn")
            nc.sync.dma_start(idn[:], idn_d[:])
            msk = cpool.tile([KB, (QC // KB) * QC], BF16, tag="msk")
            nc.sync.dma_start(msk[:], msk_d[:])
            qw = cpool.tile([128, 1], F32, tag="qw")
            nc.sync.dma_start(qw[:], qw_d[:])
            kw = cpool.tile([128, 1], F32, tag="kw")
            nc.sync.dma_start(kw[:], kw_d[:])
            ones = cpool.tile([128, 1], BF16, tag="ones")
            nc.vector.memset(ones[:], 1.0)
            b0 = cpool.tile([128, 1], F32, tag="b0")
            nc.vector.memset(b0[:], 0.0)
            bq = cpool.tile([1, 1], F32, tag="bq")
            nc.vector.memset(bq[:], float(HD * EPS))
            bk = cpool.tile([1, 1], F32, tag="bk")
            nc.vector.memset(bk[:], float(EPS))

            # full wo, staged into SBUF during phase 1 (16 contiguous panels)
            wo_sb = wopool.tile([128, NDT * NDT * 128], BF16, tag="wo")

            # long-lived activations
            qT = bigpool.tile([128, HL * S], BF16, tag="qT")
            kT = bigpool.tile([128, S], BF16, tag="kT")
            V = bigpool.tile([128, S], BF16, tag="V")   # [tok%128, blk*128+d]
            yT = bigpool.tile([128, HL * S], BF16, tag="yT")

            # ---------------- Phase 1: QKV + norm + rope ----------------
            with tc.tile_pool(name="wq", bufs=1) as wpool, \
                 tc.tile_pool(name="x", bufs=2) as xpool, \
                 tc.tile_pool(name="cs", bufs=2) as cspool, \
                 tc.tile_pool(name="scr", bufs=2) as scr, \
                 tc.tile_pool(name="smol", bufs=2) as smol, \
                 tc.tile_pool(name="p1", bufs=2, space="PSUM") as p1, \
                 tc.tile_pool(name="pss", bufs=2, space="PSUM") as pss, \
                 tc.tile_pool(name="pvt", bufs=2, space="PSUM") as pvt:

                # wqkv slice, one tile per contraction panel so the first
                # matmuls start as soon as their own panel lands
                w_ts = [wpool.tile([128, EW], BF16, tag=f"w{dt}",
                                   name=f"w{dt}") for dt in range(NDT)]

                def process_qk(ps, et, tt, cos_t, sin_t):
                    is_q = et < HL
                    # squared tile (raw, pre-normweight) -> bf16
                    sqv = smol.tile([128, TCW], BF16, tag="sq2", name="sqv")
                    nc.scalar.activation(
                        sqv[:], ps[:],
                        mybir.ActivationFunctionType.Square, bias=b0[:])
                    ss = pss.tile([1, TCW], F32, tag="ss", name="ss")
                    nc.tensor.matmul(ss[:], ones[:], sqv[:],
                                     start=True, stop=True)
                    qf = scr.tile([128, TCW], F32R, tag="qf", name="qf")
                    nc.scalar.mul(qf[:], ps[:], (qw if is_q else kw)[:])
                    sq = smol.tile([1, TCW], F32, tag="sqs", name="sq")
                    if is_q:
                        nc.scalar.activation(
                            sq[:], ss[:],
                            mybir.ActivationFunctionType.Sqrt,
                            bias=bq[:], scale=1.0)
                    else:
                        nc.scalar.activation(
                            sq[:], ss[:],
                            mybir.ActivationFunctionType.Sqrt,
                            bias=bk[:], scale=1.0 / HD)
                    inv = smol.tile([1, TCW], F32, tag="inv", name="inv")
                    nc.vector.reciprocal_approx_fast(inv[:], sq[:])
                    invb = scr.tile([128, TCW], F32, tag="invb", name="invb")
                    nc.gpsimd.partition_broadcast(invb[:], inv[:])
                    # rope: halves layout -> swap = one 32-group shuffle
                    w_s = scr.tile([128, TCW], BF16, tag="ws", name="w_s")
                    nc.vector.tensor_mul(w_s[:], qf[:], sin_t[:])
                    t2 = scr.tile([128, TCW], BF16, tag="t2", name="t2")
                    nc.vector.stream_shuffle(t2[:], w_s[:], SWAP_MASK)
                    t1 = scr.tile([128, TCW], F32, tag="t1", name="t1")
                    nc.vector.tensor_mul(t1[:], qf[:], cos_t[:])
                    nc.vector.tensor_add(t1[:], t1[:], t2[:])
                    dst = (qT[:, et * S + tt * TCW: et * S + tt * TCW + TCW]
                           if is_q else
                           kT[:, tt * TCW: tt * TCW + TCW])
                    nc.vector.tensor_mul(dst, t1[:], invb[:])

                def process_v(ps, tt):
                    vb = smol.tile([128, TCW], BF16, tag="vb", name="vb")
                    nc.scalar.copy(vb[:], ps[:])
                    for bb in range(TCW // 128):
                        tp = pvt.tile([128, 128], BF16, tag="tp", name="tp")
                        nc.tensor.transpose(
                            tp[:], vb[:, bb * 128:(bb + 1) * 128], idn[:])
                        blk = tt * (TCW // 128) + bb
                        nc.scalar.copy(V[:, blk * 128:(blk + 1) * 128], tp[:])

                pending = None  # (psum, et, tt, cos_t, sin_t)
                for tt in range(NTT):
                    xts = []
                    for dt in range(NDT):
                        if tt == 0:  # interleave weight panels in need-order
                            nc.sync.dma_start(
                                w_ts[dt][:],
                                w_d[dt * 128:(dt + 1) * 128, :])
                        xt_dt = xpool.tile([128, TCW], BF16, tag=f"x{dt}",
                                           name=f"x{dt}")
                        nc.scalar.dma_start(
                            xt_dt[:],
                            xT_d[dt * 128:(dt + 1) * 128,
                                 tt * TCW:(tt + 1) * TCW])
                        xts.append(xt_dt)
                    cos_t = cspool.tile([128, TCW], F32, tag="cos")
                    nc.sync.dma_start(cos_t[:], cos_d[:, tt * TCW:(tt + 1) * TCW])
                    sin_t = cspool.tile([128, TCW], F32, tag="sin")
                    nc.sync.dma_start(sin_t[:], sin_d[:, tt * TCW:(tt + 1) * TCW])

                    for et in range(HL + 2):
                        ps = p1.tile([128, TCW], F32, tag="ps")
                        for dt in range(NDT):
                            nc.tensor.matmul(
                                ps[:],
                                w_ts[dt][:, et * 128:(et + 1) * 128],
                                xts[dt][:],
                                start=(dt == 0), stop=(dt == NDT - 1),
                            )
                        # process the PREVIOUS tile now: its cross-engine
                        # waits overlap this tile's matmul group
                        if pending is not None:
                            pps, pet, ptt, pc, psn_ = pending
                            if pet < HL + 1:
                                process_qk(pps, pet, ptt, pc, psn_)
                            else:
                                process_v(pps, ptt)
                        pending = (ps, et, tt, cos_t, sin_t)
                pps, pet, ptt, pc, psn_ = pending
                if pet < HL + 1:
                    process_qk(pps, pet, ptt, pc, psn_)
                else:
                    process_v(pps, ptt)

            # ---------------- Phase 2: causal attention ----------------
            yfpool_cm = tc.tile_pool(name="yfp", bufs=1)
            yfpool = yfpool_cm.__enter__()
            # yf gathers the AllGather'd y slices as they arrive, per head
            yf = yfpool.tile([128, NDT * TPT], BF16, tag="yf")
            # stage the 16 wo panels now: overlaps attention, clear of the
            # phase-1 input DMA burst
            for sl in range(NDT):
                nc.scalar.dma_start(
                    wo_sb[:, sl * NDT * 128:(sl + 1) * NDT * 128],
                    wo_d[:, sl * NDT * 128:(sl + 1) * NDT * 128])
            with tc.tile_pool(name="exp", bufs=6) as epool, \
                 tc.tile_pool(name="acc", bufs=2) as accpool, \
                 tc.tile_pool(name="rs", bufs=2) as rspool, \
                 tc.tile_pool(name="rb", bufs=2) as rbpool, \
                 tc.tile_pool(name="pa", bufs=2, space="PSUM") as pa, \
                 tc.tile_pool(name="py", bufs=1, space="PSUM") as py, \
                 tc.tile_pool(name="pn", bufs=2, space="PSUM") as pn:

                pid = nc.sync.partition_id()
                goff = (pid // 4) * 512  # my batch group's 4 chunks

                for h in range(HL):
                    # g-outer sweep: each kv block's kT/V stationary serves all
                    # active q-chunks before the PE loads the next weights.
                    ps_ys = [py.tile([128, QC], F32, tag=f"y{qc}",
                                     name=f"psy{qc}")
                             for qc in range(NQC)]
                    accs = [accpool.tile([128, QC], BF16, tag=f"acc{qc}",
                                         name=f"acc{qc}")
                            for qc in range(NQC)]
                    norm_pending = []

                    def flush_norm():
                        while norm_pending:
                            qd, rsb = norm_pending.pop(0)
                            nc.vector.tensor_mul(
                                yT[:, h * S + qd * QC: h * S + (qd + 1) * QC],
                                ps_ys[qd][:], rsb[:])

                    for g in range(NB):
                        qcs = [qc for qc in range(NQC) if g < 4 * (qc + 1)]
                        flush_norm()
                        exs = {}
                        for qc in qcs:
                            diag = (qc == g // 4)
                            t = g % 4
                            c0 = t * 128 if (diag and DIAG_SLICE) else 0
                            ps_s = pa.tile([KB, QC], F32, tag="s")
                            nc.tensor.matmul(
                                ps_s[:, c0:],
                                kT[:, g * KB:(g + 1) * KB],
                                qT[:, h * S + qc * QC + c0:
                                   h * S + (qc + 1) * QC],
                                start=True, stop=True)
                            ex = epool.tile([KB, QC], BF16, tag="e")
                            nc.scalar.activation(
                                ex[:, c0:], ps_s[:, c0:],
                                mybir.ActivationFunctionType.Exp,
                                bias=b0[0:KB, :])
                            if diag:  # diagonal region: causal mask
                                nc.vector.tensor_mul(
                                    ex[:, c0:], ex[:, c0:],
                                    msk[:, t * QC + c0:(t + 1) * QC])
                            exs[qc] = (ex, c0)
                        for qc in qcs:
                            ex, c0 = exs[qc]
                            nc.tensor.matmul(
                                ps_ys[qc][:, c0:],
                                V[:, g * 128:(g + 1) * 128],
                                ex[:, c0:],
                                start=(g == 0), stop=(g == 4 * qc + 3))
                        for qc in qcs:
                            ex, c0 = exs[qc]
                            if g == 0:
                                nc.vector.tensor_copy(accs[qc][:], ex[:])
                            else:
                                nc.vector.tensor_add(
                                    accs[qc][:, c0:], accs[qc][:, c0:],
                                    ex[:, c0:])
                            if g == 4 * qc + 3:
                                # denominator: one tiny f32r ones-matmul
                                psn = pn.tile([1, QC], F32, tag="n",
                                              name="psn")
                                nc.tensor.matmul(
                                    psn[:], ones[:], accs[qc][:],
                                    start=True, stop=True)
                                rs = rspool.tile([1, QC], F32, tag="r",
                                                 name="rs")
                                nc.vector.reciprocal_approx_fast(rs[:], psn[:])
                                rsb = rbpool.tile([128, QC], F32, tag="rb",
                                                  name="rsb")
                                nc.gpsimd.partition_broadcast(rsb[:], rs[:])
                                norm_pending.append((qc, rsb))
                    flush_norm()

                    # per-head 8-core AllToAll re-shards y from head-split
                    # to token-split: chunk j carries token-quarter j%4, so
                    # every core writes the same 8 chunks (rank-uniform graph)
                    # and reads back its group's 4 chunks by a partition_id-
                    # derived row offset. 875KB wire/rank vs 1.5MB for a
                    # group-local AllGather.
                    in_b = dpool.tile([8 * 128, TPT], BF16, tag=f"a2i{h}")
                    out_b = dpool.tile([8 * 128, TPT], BF16, tag=f"a2o{h}")
                    for j in range(8):
                        nc.scalar.dma_start(
                            in_b[j * 128:(j + 1) * 128, :],
                            yT[:, h * S + (j % 4) * TPT:
                               h * S + (j % 4 + 1) * TPT])
                    nc.gpsimd.collective_compute(
                        "AllToAll", mybir.AluOpType.bypass,
                        replica_groups=[[0, 1, 2, 3, 4, 5, 6, 7]],
                        ins=[in_b.opt()], outs=[out_b.opt()])
                    for r in range(4):
                        et = 4 * r + h
                        nc.sync.dma_start(
                            yf[:, et * TPT:(et + 1) * TPT],
                            out_b[bass.ds(goff + r * 128, 128), :])

            # ---------------- output projection (two passes) ----------------
            # Pass A accumulates heads 0..2 (available before the last
            # AllGather) into bf16 partials; pass B adds head 3's contribution
            # as soon as its gather lands. wo panels already staged in SBUF.
            with tc.tile_pool(name="part", bufs=1) as partpool, \
                 tc.tile_pool(name="ot", bufs=2) as otpool, \
                 tc.tile_pool(name="po", bufs=2, space="PSUM") as po:
                part = partpool.tile([128, NDT * TPT], BF16, tag="part")
                etsA = [4 * r + hl for hl in range(HL - 1) for r in range(4)]
                etsB = [4 * r + (HL - 1) for r in range(4)]
                for ot in range(NDT):
                    ps_o = po.tile([128, TPT], F32, tag="o")
                    for i, et in enumerate(etsA):
                        nc.tensor.matmul(
                            ps_o[:],
                            wo_sb[:, (ot * NDT + et) * 128:
                                  (ot * NDT + et + 1) * 128],
                            yf[:, et * TPT:(et + 1) * TPT],
                            start=(i == 0), stop=(i == len(etsA) - 1))
                    nc.scalar.copy(part[:, ot * TPT:(ot + 1) * TPT], ps_o[:])
                for ot in range(NDT):
                    ps_o = po.tile([128, TPT], F32, tag="o")
                    for i, et in enumerate(etsB):
                        nc.tensor.matmul(
                            ps_o[:],
                            wo_sb[:, (ot * NDT + et) * 128:
                                  (ot * NDT + et + 1) * 128],
                            yf[:, et * TPT:(et + 1) * TPT],
                            start=(i == 0), stop=(i == len(etsB) - 1))
                    ott = otpool.tile([128, TPT], BF16, tag="ot")
                    nc.vector.tensor_add(ott[:], ps_o[:],
                                         part[:, ot * TPT:(ot + 1) * TPT])
                    nc.sync.dma_start(out_d[ot * 128:(ot + 1) * 128, :], ott[:])
            yfpool_cm.__exit__(None, None, None)

    nc.compile()
    return nc


def make_in_maps(x, freqs_cis, wqkv, wo, q_norm_w, k_norm_w, S):
    """Host-side sharding / layout prep. Returns list of 8 input dicts."""
    bf = ml_dtypes.bfloat16
    QC = 512
    KB = 128

    # head_dim permutation: stream_shuffle permutes within 32-partition
    # quadrants, so quadrant s holds pairs 16s..16s+15 as [16 evens, 16 odds];
    # rope's pair swap is then a 16-offset swap inside every quadrant
    perm = np.zeros(HD, np.int64)
    for s_ in range(4):
        for i_ in range(16):
            perm[s_ * 32 + i_] = 2 * (16 * s_ + i_)
            perm[s_ * 32 + 16 + i_] = 2 * (16 * s_ + i_) + 1

    # rope tables in the same layout; sin rows for odd dims carry -sin (sign
    # pre-folded before the quadrant swap)
    cos = np.asarray(freqs_cis[:S, :, 0], np.float32)   # [S, 64]
    sin = np.asarray(freqs_cis[:S, :, 1], np.float32)
    cosF = np.zeros((HD, S), np.float32)
    sinF = np.zeros((HD, S), np.float32)
    for s_ in range(4):
        for i_ in range(16):
            p_ = 16 * s_ + i_
            cosF[s_ * 32 + i_] = cos[:, p_]
            cosF[s_ * 32 + 16 + i_] = cos[:, p_]
            sinF[s_ * 32 + i_] = sin[:, p_]
            sinF[s_ * 32 + 16 + i_] = -sin[:, p_]
    cosF = np.ascontiguousarray(cosF)
    sinF = np.ascontiguousarray(sinF)

    ident = np.eye(128, dtype=bf)

    # masks [KB, (QC//KB)*QC]: pattern t for the t-th kv block inside the
    # diagonal QC-region: allowed iff (t*KB + r) <= c
    r = np.arange(KB)[:, None]
    c = np.arange(QC)[None, :]
    pats = [((t * KB + r) <= c).astype(np.float32) for t in range(QC // KB)]
    masks = np.concatenate(pats, axis=1).astype(bf)

    qwv = np.asarray(q_norm_w, np.float32)[perm].reshape(128, 1)
    kwv = np.asarray(k_norm_w, np.float32)[perm].reshape(128, 1)

    # wo panels: woS[p, (ot*16+a)*128+o] = wo[ot*128+o, a*128+p] so every
    # output tile's weight panel is one contiguous per-partition run
    wof = np.asarray(wo, np.float32)
    woS = np.ascontiguousarray(
        wof.reshape(NDT, 128, NDT, 128).transpose(3, 0, 2, 1)
        .reshape(128, NDT * NDT * 128)).astype(bf)

    xTb = []
    for b in range(2):
        xTb.append(np.ascontiguousarray(np.asarray(x[b], np.float32).T)
                   .astype(bf))

    wq = np.asarray(wqkv, np.float32)
    q_sz = NH * HD
    in_maps = []
    for c_id in range(N_CORES):
        b, rk = c_id // 4, c_id % 4
        blocks = []
        for hq in range(HL):  # 4 q heads, rows permuted to halves layout
            blk = wq[(rk * HL + hq) * HD:(rk * HL + hq + 1) * HD]
            blocks.append(blk[perm])
        kblk = wq[q_sz + rk * HD: q_sz + (rk + 1) * HD]
        blocks.append(kblk[perm])
        blocks.append(wq[q_sz + NKV * HD + rk * HD:
                         q_sz + NKV * HD + (rk + 1) * HD])  # v head, unpermuted
        rows = np.concatenate(blocks, axis=0)               # [768, 2048]
        wslice = np.ascontiguousarray(rows.T).astype(bf)    # [2048, 768]
        in_maps.append({
            "xT": xTb[b], "wslice": wslice, "woS": woS,
            "cosF": cosF, "sinF": sinF,
            "ident": ident, "masks": masks,
            "qw": qwv, "kw": kwv,
        })
    return in_maps


_NC_CACHE = {}


def kernel(x, freqs_cis, mask, wqkv, wo, q_norm_w, k_norm_w):
    x = np.asarray(x)
    S = x.shape[1]
    if S not in _NC_CACHE:
        _NC_CACHE[S] = build_graph(S)
    nc = _NC_CACHE[S]
    in_maps = make_in_maps(x, freqs_cis, wqkv, wo, q_norm_w, k_norm_w, S)
    res = run_bass_kernel_spmd(nc, in_maps, core_ids=list(range(N_CORES)))
    TPT = S // 4
    out = np.empty((2, S, DIM), np.float32)
    for c_id in range(N_CORES):
        b, rk = c_id // 4, c_id % 4
        out[b, rk * TPT:(rk + 1) * TPT, :] = res.results[c_id]["out"].T.astype(np.float32)
    return out
